# revision 95
# baseline (speedup 1.0000x reference)
"""Trainium2 Bass kernel for nn_ActionNet (Wigner-D block-diag rotation + dense +
4x stride-2 conv_transpose decoder), data-parallel over 8 NeuronCores.

Math: real Wigner D^l(a,b,g) = Zr(a) @ dr(b) @ Zr(g), with
  Zr(t): Zr[m,m]=cos(mt), Zr[l+m,l-m]=sin(mt), Zr[l-m,l+m]=-sin(mt)
  dr(b)[u,v] = sum_q Cr_l[u,v,q] cos(b/2)^(2l-q) sin(b/2)^q,  Cr_l = Re(B C_l B^H)
conv_transpose(s=2,k=4,SAME) phases (verified vs jax):
  out[2p+d]: d=0 -> K[2] x[p] + K[0] x[p-1];  d=1 -> K[1] x[p] + K[3] x[p+1]
Trig: sx = sin(t/2 - pi/2) (safe LUT domain), cx = sqrt(1-sx^2);
  cos(t/2) = -sx, sin(t/2) = cx; then double-angle + recurrence for cos/sin(m t).
"""
import math
import sys
import types

import numpy as np

sys.path.insert(0, '/opt/trn_rl_repo')
import ml_dtypes

DEGREES = 6
NL = DEGREES + 1
R = 10
N_BATCH = 2048
NCORES = 8
NPC = N_BATCH // NCORES
NT = 32
PI = math.pi
TAPS = {0: [(2, 0), (0, -1)], 1: [(1, 0), (3, 1)]}
OFF49 = [l * l for l in range(NL + 1)]
OFF455 = np.cumsum([0] + [(2 * l + 1) ** 2 for l in range(NL)]).tolist()


def _install_axon_shim():
    if 'antenv.axon_hooks' in sys.modules:
        return
    mod = types.ModuleType('antenv.axon_hooks')
    _h = [None]
    mod.set_axon_ntff_profile_hook = lambda h: _h.__setitem__(0, h)
    mod.get_axon_ntff_profile_hook = lambda: _h[0]
    sys.modules['antenv.axon_hooks'] = mod
    try:
        import antenv
        antenv.axon_hooks = mod
        from trn_agent_boot.trn_boot import _ntff_profile_via_ctypes
        mod.set_axon_ntff_profile_hook(_ntff_profile_via_ctypes('/opt/axon/libaxon_pjrt.so'))
    except Exception:
        pass


def _wigner_coeffs(l):
    f = math.factorial
    n = 2 * l + 1
    C = np.zeros((n, n, n))
    for mp in range(-l, l + 1):
        for m in range(-l, l + 1):
            pref = math.sqrt(f(l + mp) * f(l - mp) * f(l + m) * f(l - m))
            for s in range(max(0, m - mp), min(l + m, l - mp) + 1):
                q = mp - m + 2 * s
                den = f(l + m - s) * f(s) * f(mp - m + s) * f(l - mp - s)
                C[mp + l, m + l, q] += ((-1.0) ** (mp - m + s)) * pref / den
    return C


def _real_basis(l):
    n = 2 * l + 1
    B = np.zeros((n, n), dtype=np.complex128)
    B[l, l] = 1.0
    isq = 1.0 / math.sqrt(2.0)
    for m in range(1, l + 1):
        B[l + m, l + m] = ((-1) ** m) * isq
        B[l + m, l - m] = isq
        B[l - m, l - m] = 1j * isq
        B[l - m, l + m] = -1j * ((-1) ** m) * isq
    return B


_CSZ = [49 - OFF49[m] for m in range(7)]
_C_OFF = [0]
for _m in range(6):
    _C_OFF.append(_C_OFF[-1] + _CSZ[_m])
_S_OFF = [252]
for _m in range(1, 6):
    _S_OFF.append(_S_OFF[-1] + _CSZ[_m])


def _fcol(f):
    # feature index -> F column (trig block occupies F cols 384..397)
    return f if f < 384 else f + 14


def _build_k(rep):
    """Fold d(b) coeffs, Zg(g) trig selection and item_rep into constant
    matmul weights: t2 = F @ K1, t2flip = F @ K2, item = caE*t2 + saE*t2flip
    with caE/saE = F[:,384:398] @ Ecs."""
    K1 = np.zeros((512, 490), np.float64)
    Ecs = np.zeros((2, 14, 490), np.float32)
    for l in range(NL):
        n = 2 * l + 1
        C = _wigner_coeffs(l)
        B = _real_basis(l)
        Cr = np.real(np.einsum('ua,abq,vb->uvq', B, C, B.conj()))
        for q in range(n):
            pcol = OFF49[l] + q
            for m in range(l + 1):
                fc = _C_OFF[m] + pcol - OFF49[m]
                for u in range(n):
                    col = (OFF49[l] + u) * R
                    for v in {l - m, l + m}:
                        K1[_fcol(fc), col:col + R] += Cr[u, v, q] * rep[OFF49[l] + v]
            for m in range(1, l + 1):
                fs = _S_OFF[m - 1] + pcol - OFF49[m]
                for u in range(n):
                    col = (OFF49[l] + u) * R
                    for v in (l - m, l + m):
                        K1[_fcol(fs), col:col + R] += \
                            np.sign(v - l) * Cr[u, v, q] * rep[OFF49[l] + 2 * l - v]
        for u in range(n):
            m = abs(u - l)
            col = (OFF49[l] + u) * R
            Ecs[0, m, col:col + R] = 1.0
            if u != l:
                Ecs[1, 7 + m, col:col + R] = float(np.sign(u - l))
    K2 = np.zeros_like(K1)
    for l in range(NL):
        for u in range(2 * l + 1):
            K2[:, (OFF49[l] + u) * R:(OFF49[l] + u + 1) * R] = \
                K1[:, (OFF49[l] + 2 * l - u) * R:(OFF49[l] + 2 * l - u + 1) * R]
    return K1.astype(np.float32), K2.astype(np.float32), Ecs


def _flip49(x):
    out = np.empty_like(x)
    for l in range(NL):
        out[OFF49[l]:OFF49[l + 1]] = x[OFF49[l]:OFF49[l + 1]][::-1]
    return out


def _ki(d, s):
    for k, ss in TAPS[d]:
        if ss == s:
            return k
    return None


def _sis(d):
    return [s for s in (-1, 0, 1) if _ki(d, s) is not None]


def _prep_weights(item_rep, W, b, k1, b1, k2, b2, k3, b3, k4, b4):
    bf16 = ml_dtypes.bfloat16
    inp = {}
    K1, K2, Ecs = _build_k(item_rep.astype(np.float64))
    inp['K1'] = K1.reshape(4, 128, 490)
    inp['K2'] = K2.reshape(4, 128, 490)
    inp['Ecs'] = Ecs
    inp['ident'] = np.eye(128, dtype=np.float32)
    Wp = np.zeros((512, 4096), np.float32)
    Wp[:490] = W
    inp['Wb'] = np.ascontiguousarray(Wp.reshape(4, 128, 4096)).astype(bf16)
    inp['bd'] = np.ascontiguousarray(b.reshape(32, 128).T).astype(np.float32)
    k1s = np.zeros((128, 4, 4, 2, 128), np.float32)
    for pi4, (di, dj) in enumerate([(0, 0), (0, 1), (1, 0), (1, 1)]):
        tn = 0
        for si in _sis(di):
            for sj in _sis(dj):
                kk = k1[_ki(di, si), _ki(dj, sj)]
                for h in range(2):
                    k1s[:, pi4, tn, h, :] = kk[h * 128:(h + 1) * 128]
                tn += 1
    inp['k1s'] = k1s.astype(bf16)
    inp['b1'] = b1.reshape(128, 1).astype(np.float32)
    # odd-pair outputs: psum col q' computes {2q'+1 (dj1), 2q'+2 (dj0)} whose
    # input windows coincide at {x[q'], x[q'+1]} -> 4 zero-waste accums
    k2s = np.zeros((128, 2, 2, 2, 128), np.float32)
    for di in range(2):
        for ii, si in enumerate(_sis(di)):
            for p in range(2):
                k2s[:, di, ii, p, 0:64] = k2[_ki(di, si), 0 if p == 0 else 2]
                k2s[:, di, ii, p, 64:128] = k2[_ki(di, si), 1 if p == 0 else 3]
    inp['k2s'] = k2s.astype(bf16)
    # edge outputs j=0 (dj0 half: K[2]x[0]) and j=15 (dj1 half: K[1]x[7]);
    # 64-wide lhsT halves the weight-load cost of these tiny matmuls
    k2e = np.zeros((128, 2, 2, 2, 64), np.float32)
    for di in range(2):
        for ii, si in enumerate(_sis(di)):
            k2e[:, di, ii, 0, :] = k2[_ki(di, si), 2]
            k2e[:, di, ii, 1, :] = k2[_ki(di, si), 1]
    inp['k2e'] = k2e.astype(bf16)
    inp['b2'] = np.tile(b2, 2).reshape(128, 1).astype(np.float32)
    # x2 stored as odd-base j-pairs {2q'+1, 2q'+2}; partition row parity
    # dj_row = position parity. Only 2 j-accums (delta) per output quad.
    k3s = np.zeros((2, 64, 2, 2, 2, 128), np.float32)
    for di in range(2):
        for ii, si in enumerate(_sis(di)):
            for delta in range(2):
                for dj_row in range(2):
                    for jm in range(4):
                        dj_out = jm & 1
                        s = 2 * delta - dj_row - (jm >> 1)
                        if s in (-1, 0, 1):
                            kj = _ki(dj_out, s)
                            if kj is not None:
                                k3s[dj_row, :, di, ii, delta, jm * 32:(jm + 1) * 32] = \
                                    k3[_ki(di, si), kj]
    inp['k3s'] = k3s.reshape(128, 2, 2, 2, 128).astype(bf16)
    inp['b3'] = np.tile(b3, 4).reshape(128, 1).astype(np.float32)
    # conv4 m-col = di*64 + dj*32 + jout
    k4s = np.zeros((4, 32, 8, 3, 128), np.float32)
    for bblk in range(8):
        for ii, si in enumerate((-1, 0, 1)):
            for jm in range(4):
                j = 4 * bblk + jm
                for sj in (-1, 0, 1):
                    jout = j - sj
                    if not (0 <= jout < 32):
                        continue
                    for di in range(2):
                        ki = _ki(di, si)
                        if ki is None:
                            continue
                        for dj in range(2):
                            kj = _ki(dj, sj)
                            if kj is not None:
                                k4s[jm, :, bblk, ii, di * 64 + 2 * jout + dj] = k4[ki, kj, :, 0]
    inp['k4s'] = k4s.reshape(128, 8, 3, 128).astype(bf16)
    inp['b4'] = np.full((128, 1), float(b4[0]), np.float32)
    return inp


def _build():
    import concourse.bass as bass
    import concourse.mybir as mybir
    import concourse.tile as tile
    from concourse import bacc
    import contextlib

    dt = mybir.dt
    AF = mybir.ActivationFunctionType
    ALU = mybir.AluOpType
    f32, f32r, bf16 = dt.float32, dt.float32r, dt.bfloat16
    nc = bacc.Bacc("TRN2", target_bir_lowering=False, debug=False, num_devices=NCORES)

    def din(name, shape, dtype=f32):
        return nc.dram_tensor(name, list(shape), dtype, kind="ExternalInput").ap()

    ang = din('angles', [NPC, 3])
    K1_d = din('K1', [4, 128, 490], f32r)
    K2_d = din('K2', [4, 128, 490], f32r)
    Ecs_d = din('Ecs', [2, 14, 490], f32r)
    id_d = din('ident', [128, 128])
    Wb_d = din('Wb', [4, 128, 4096], bf16)
    bd_d = din('bd', [128, 32])
    k1s_d = din('k1s', [128, 4, 4, 2, 128], bf16)
    b1_d = din('b1', [128, 1])
    k2s_d = din('k2s', [128, 2, 2, 2, 128], bf16)
    k2e_d = din('k2e', [128, 2, 2, 2, 64], bf16)
    b2_d = din('b2', [128, 1])
    k3s_d = din('k3s', [128, 2, 2, 2, 128], bf16)
    b3_d = din('b3', [128, 1])
    k4s_d = din('k4s', [128, 8, 3, 128], bf16)
    b4_d = din('b4', [128, 1])
    out_d = nc.dram_tensor('out', [NPC, 64, 64], f32, kind="ExternalOutput").ap()

    def mk(t, off, dims):
        a = t[:]
        return bass.AP(tensor=a.tensor, offset=a.offset + off,
                       ap=[[a.ap[0][0], a.ap[0][1]]] + [[s, c] for s, c in dims])

    def mkh(t, p0, pc, off, dims):
        # partition-ranged variant: partitions [p0, p0+pc)
        a = t[:]
        ps = a.ap[0][0]
        return bass.AP(tensor=a.tensor, offset=a.offset + p0 * ps + off,
                       ap=[[ps, pc]] + [[s, c] for s, c in dims])

    with tile.TileContext(nc) as tc:
        ctx = contextlib.ExitStack()
        wp = ctx.enter_context(tc.tile_pool(name="wts", bufs=1))
        apl = ctx.enter_context(tc.tile_pool(name="acts", bufs=1))
        tp = ctx.enter_context(tc.tile_pool(name="tmp", bufs=1))
        pp = ctx.enter_context(tc.tile_pool(name="ps", bufs=8, space="PSUM"))

        def psum():
            return pp.tile([128, 512], f32, tag="ps", name="ps")

        def load(dram_ap, shape, dtype=f32, tag=None):
            t = wp.tile(shape, dtype, tag=tag)
            nc.sync.dma_start(out=t[:], in_=dram_ap)
            return t

        # angles for both 128-sample blocks side by side: cols (s0:a,b,g, s1:a,b,g)
        a6 = tp.tile([128, 6], f32, tag="a6", name="a6")
        nc.sync.dma_start(out=a6[:, 0:3], in_=ang[0:128, :])
        nc.sync.dma_start(out=a6[:, 3:6], in_=ang[128:256, :])
        ident = load(id_d[:, :], [128, 128], tag="ident")

        c_zero = wp.tile([128, 1], f32, tag="c_zero", name="c_zero")
        nc.vector.memset(c_zero[:], 0.0)
        c_half = wp.tile([128, 1], f32, tag="c_half", name="c_half")
        c_nhpi = wp.tile([128, 1], f32, tag="c_nhpi", name="c_nhpi")
        c_none = wp.tile([128, 1], f32, tag="c_none", name="c_none")
        c_one = wp.tile([128, 1], f32, tag="c_one", name="c_one")
        nc.vector.memset(c_half[:], 0.5)
        nc.vector.memset(c_nhpi[:], -PI / 2.0)
        nc.vector.memset(c_none[:], -1.0)
        nc.vector.memset(c_one[:], 1.0)

        # y-stage weights, split so matmul lhsT/rhs share base partition 0
        wdp = tc.tile_pool(name="wdense", bufs=1)
        wdpo = wdp.__enter__()
        def loadw(dram_ap, shape, dtype=f32, tag=None):
            t = wdpo.tile(shape, dtype, tag=tag, name=tag)
            nc.sync.dma_start(out=t[:], in_=dram_ap)
            return t
        K1t = [loadw(K1_d[c, :, :], [128, 490], f32r, tag=f"K1{c}") for c in range(4)]
        K2t = [loadw(K2_d[c, :, :], [128, 490], f32r, tag=f"K2{c}") for c in range(4)]
        Eca = loadw(Ecs_d[0, :, :], [14, 490], f32r, tag="Eca")
        Esa = loadw(Ecs_d[1, :, :], [14, 490], f32r, tag="Esa")
        # conv/dense weights after the wigner constants on the DMA queue
        bd = load(bd_d[:, :], [128, 32], tag="bd")
        k1s = load(k1s_d[:, :, :, :, :], [128, 4, 4, 2, 128], bf16, tag="k1s")
        b1 = load(b1_d[:, :], [128, 1], tag="b1")
        k2s = load(k2s_d[:, :, :, :, :], [128, 2, 2, 2, 128], bf16, tag="k2s")
        k2e = load(k2e_d[:, :, :, :, :], [128, 2, 2, 2, 64], bf16, tag="k2e")
        b2 = load(b2_d[:, :], [128, 1], tag="b2")
        k3s = load(k3s_d[:, :, :, :, :], [128, 2, 2, 2, 128], bf16, tag="k3s")
        b3 = load(b3_d[:, :], [128, 1], tag="b3")
        k4s = load(k4s_d[:, :, :, :], [128, 8, 3, 128], bf16, tag="k4s")
        b4 = load(b4_d[:, :], [128, 1], tag="b4")

        itemTb = [wdpo.tile([128, 256], bf16, tag=f"itemTb{kc}", name=f"itemTb{kc}") for kc in range(4)]
        nc.vector.memset(itemTb[3][:], 0.0)

        # dense inputs/outputs staged up-front so each 128-sample block can
        # flow wigner -> dense -> convs without waiting for the other block
        c1in = [apl.tile([128, NPC, 6, 6], bf16, tag=f"c1in{h}", name=f"c1in{h}") for h in range(2)]
        for h in range(2):
            nc.gpsimd.memset(mk(c1in[h], 0, [(36, NPC), (30, 2), (1, 6)]), 0.0)
            nc.gpsimd.memset(mk(c1in[h], 6, [(36, NPC), (6, 4), (5, 2)]), 0.0)
        Wk = []
        for kc in range(4):
            wt = wdpo.tile([128, 4096], bf16, tag=f"Wk{kc}", name=f"Wk{kc}")
            nc.sync.dma_start(out=wt[:], in_=Wb_d[kc, :, :])
            Wk.append(wt)

        # ===== Wigner trig/features for BOTH blocks =====
        # All chain ops run 6 columns wide (a,b,g x s0,s1) — narrower DVE ops
        # hit a ~5us slow path. Unused b-trig / a,g-power columns are junk.
        # sx = sin(t/2 - pi/2), cx = sqrt(1 - sx^2)
        sx = tp.tile([128, 6], f32, tag="sx", name="sx")
        cx = tp.tile([128, 6], f32, tag="cx", name="cx")
        sq = tp.tile([128, 6], f32, tag="sqt", name="sqt")
        nc.scalar.activation(sx[:], a6[:], AF.Sin, bias=c_nhpi[:], scale=c_half[:])
        # keep the whole prefix on scalar: avoids two cross-engine joins
        nc.scalar.activation(sq[:], sx[:], AF.Square, bias=c_zero[:])
        nc.scalar.activation(cx[:], sq[:], AF.Sqrt, bias=c_one[:], scale=c_none[:])
        ngx = tp.tile([128, 6], f32, tag="ngx", name="ngx")  # cos(t/2) = -sx
        nc.scalar.activation(ngx[:], sx[:], AF.Identity, bias=c_zero[:],
                             scale=c_none[:])
        # cos/sin(m t): 6 groups of 7 cols, (s0:a,b,g, s1:a,b,g)
        cosT = tp.tile([128, 42], f32, tag="cosT", name="cosT")
        sinT = tp.tile([128, 42], f32, tag="sinT", name="sinT")
        nc.vector.memset(mk(cosT, 0, [(7, 6)]), 1.0)
        nc.vector.memset(mk(sinT, 0, [(7, 6)]), 0.0)
        t0 = tp.tile([128, 6], f32, tag="t0", name="t0")
        nc.vector.tensor_mul(t0[:], sx[:], sx[:])
        c1 = tp.tile([128, 6], f32, tag="c1", name="c1")
        s1 = tp.tile([128, 6], f32, tag="s1", name="s1")
        nc.vector.tensor_scalar(c1[:], t0[:], 2.0, -1.0, op0=ALU.mult, op1=ALU.add)
        nc.vector.tensor_mul(t0[:], cx[:], sx[:])
        nc.vector.tensor_scalar_mul(s1[:], t0[:], -2.0)  # s1 = sin(t)
        nc.vector.tensor_copy(mk(cosT, 1, [(7, 6)]), c1[:])
        nc.vector.tensor_copy(mk(sinT, 1, [(7, 6)]), s1[:])
        ta = tp.tile([128, 6], f32, tag="ta", name="ta")
        tb = tp.tile([128, 6], f32, tag="tb", name="tb")
        for m in range(2, 7):
            pcm = mk(cosT, m - 1, [(7, 6)])
            psm = mk(sinT, m - 1, [(7, 6)])
            nc.vector.tensor_mul(ta[:], pcm, c1[:])
            nc.vector.tensor_mul(tb[:], psm, s1[:])
            nc.vector.tensor_sub(mk(cosT, m, [(7, 6)]), ta[:], tb[:])
            nc.vector.tensor_mul(ta[:], psm, c1[:])
            nc.vector.tensor_mul(tb[:], pcm, s1[:])
            nc.vector.tensor_add(mk(sinT, m, [(7, 6)]), ta[:], tb[:])
        # power tables: 6 groups of 13; only the b-groups (offset 13, 13+39)
        # are consumed by P below
        cpow = tp.tile([128, 78], f32, tag="cpow", name="cpow")
        spow = tp.tile([128, 78], f32, tag="spow", name="spow")
        for pw, base in ((cpow, ngx), (spow, cx)):
            nc.vector.memset(mk(pw, 0, [(13, 6)]), 1.0)
            nc.vector.tensor_copy(mk(pw, 1, [(13, 6)]), base[:])
            xw = tp.tile([128, 6], f32, tag="xw", name="xw")
            nc.vector.tensor_mul(xw[:], base[:], base[:])
            nc.vector.tensor_mul(mk(pw, 2, [(13, 6), (1, 2)]),
                                 mk(pw, 0, [(13, 6), (1, 2)]),
                                 mk(xw, 0, [(1, 6), (0, 2)]))
            nc.vector.tensor_mul(xw[:], xw[:], xw[:])
            nc.vector.tensor_mul(mk(pw, 4, [(13, 6), (1, 4)]),
                                 mk(pw, 0, [(13, 6), (1, 4)]),
                                 mk(xw, 0, [(1, 6), (0, 4)]))
            nc.vector.tensor_mul(xw[:], xw[:], xw[:])
            nc.vector.tensor_mul(mk(pw, 8, [(13, 6), (1, 5)]),
                                 mk(pw, 0, [(13, 6), (1, 5)]),
                                 mk(xw, 0, [(1, 6), (0, 5)]))
        # P cols s*49: P[:, (l,q)] = cos(b/2)^(2l-q) sin(b/2)^q
        Pt = tp.tile([128, 98], f32, tag="P2", name="P2")
        for l in range(NL):
            n = 2 * l + 1
            nc.vector.tensor_mul(mk(Pt, OFF49[l], [(49, 2), (1, n)]),
                                 mk(cpow, 13 + 2 * l, [(39, 2), (-1, n)]),
                                 mk(spow, 13, [(39, 2), (1, n)]))
        # F cols s*512: product features P*cos(m g) etc; trig(a) at 384..397
        Ft = tp.tile([128, 1024], f32, tag="F2", name="F2")
        nc.vector.memset(mk(Ft, 469, [(512, 2), (1, 43)]), 0.0)

        def emit_feat(fstart, size, pcol0, trig, tcol):
            def one(fs, pc, sz):
                off = 0 if fs < 384 else 14
                nc.vector.tensor_mul(mk(Ft, fs + off, [(512, 2), (1, sz)]),
                                     mk(Pt, pc, [(49, 2), (1, sz)]),
                                     mk(trig, tcol, [(21, 2), (0, sz)]))
            if fstart + size <= 384 or fstart >= 384:
                one(fstart, pcol0, size)
            else:
                k = 384 - fstart
                one(fstart, pcol0, k)
                one(384, pcol0 + k, size - k)

        for m in range(7):
            emit_feat(_C_OFF[m], 49 - OFF49[m], OFF49[m], cosT, 14 + m)
        for m in range(1, 7):
            emit_feat(_S_OFF[m - 1], 49 - OFF49[m], OFF49[m], sinT, 14 + m)
        nc.vector.tensor_copy(mk(Ft, 384, [(512, 2), (1, 7)]),
                              mk(cosT, 0, [(21, 2), (1, 7)]))
        nc.vector.tensor_copy(mk(Ft, 391, [(512, 2), (1, 7)]),
                              mk(sinT, 0, [(21, 2), (1, 7)]))

        # ================= per-block: transpose, K matmuls, item, dense =====
        for s in range(2):
            # transpose F -> 4 base-0 lhsT chunks
            FTs = []
            for c in range(4):
                ptp = psum()
                nc.tensor.transpose(ptp[0:128, 0:128],
                                    Ft[:, s * 512 + c * 128:s * 512 + (c + 1) * 128],
                                    ident[:])
                ft = tp.tile([128, 128], f32r, tag=f"FT{s}{c}", name=f"FT{s}{c}")
                nc.vector.tensor_copy(ft[:], ptp[0:128, 0:128])
                FTs.append(ft)
            # t2 = F @ K1, t2flip = F @ K2, envelopes via selector matmuls
            pt2 = psum()
            for c in range(4):
                nc.tensor.matmul(pt2[:, 0:490], FTs[c][:], K1t[c][:],
                                 start=(c == 0), stop=(c == 3))
            pt2f = psum()
            for c in range(4):
                nc.tensor.matmul(pt2f[:, 0:490], FTs[c][:], K2t[c][:],
                                 start=(c == 0), stop=(c == 3))
            pca = psum()
            nc.tensor.matmul(pca[:, 0:490], FTs[3][0:14, :], Eca[:],
                             start=True, stop=True)
            psa = psum()
            nc.tensor.matmul(psa[:, 0:490], FTs[3][0:14, :], Esa[:],
                             start=True, stop=True)
            caE = tp.tile([128, 490], f32, tag=f"caE{s}", name=f"caE{s}")
            saE = tp.tile([128, 490], f32, tag=f"saE{s}", name=f"saE{s}")
            nc.scalar.activation(caE[:], pca[:, 0:490], AF.Identity, bias=c_zero[:])
            nc.scalar.activation(saE[:], psa[:, 0:490], AF.Identity, bias=c_zero[:])
            item = tp.tile([128, 512], f32, tag=f"item{s}", name=f"item{s}")
            tmp2 = tp.tile([128, 490], f32, tag=f"tmpi{s}", name=f"tmpi{s}")
            nc.vector.tensor_mul(item[:, 0:490], caE[:], pt2[:, 0:490])
            nc.vector.tensor_mul(tmp2[:], saE[:], pt2f[:, 0:490])
            nc.vector.tensor_add(item[:, 0:490], item[:, 0:490], tmp2[:])
            for kc in range(4):
                cnt = 128 if kc < 3 else 106
                pit = psum()
                nc.tensor.transpose(pit[0:cnt, 0:128], item[:, kc * 128:kc * 128 + cnt],
                                    ident[:])
                nc.vector.tensor_copy(itemTb[kc][0:cnt, s * 128:(s + 1) * 128],
                                      pit[0:cnt, 0:128])

            # ===== dense for this 128-sample block -> c1in =====
            for yy in range(4):
                for xp in range(2):
                    pd2 = psum()
                    for xh in range(4):
                        xx = xp * 2 + xh // 2
                        h = xh % 2
                        mc = yy * 8 + xx * 2 + h
                        for kc in range(4):
                            nc.tensor.matmul(pd2[:, xh * 128:(xh + 1) * 128],
                                             Wk[kc][:, mc * 128:(mc + 1) * 128],
                                             itemTb[kc][:, s * 128:(s + 1) * 128],
                                             start=(kc == 0), stop=(kc == 3))
                        nc.scalar.activation(
                            mk(c1in[h], s * 128 * 36 + (1 + yy) * 6 + (1 + xx),
                               [(36, 128)]),
                            pd2[:, xh * 128:(xh + 1) * 128], AF.Relu,
                            bias=bd[:, mc:mc + 1])

        wdp.__exit__(None, None, None)
        # ================= conv stack, n-tiles of 32 =================
        c1os = [apl.tile([128, NT, 10, 10], bf16, tag=f"c1o{i}", name=f"c1o{i}")
                for i in range(2)]
        c2o = apl.tile([128, NT, 18, 10], bf16, tag="c2o", name="c2o")
        c3o = apl.tile([128, NT, 8, 34], bf16, tag="c3o", name="c3o")
        otile = apl.tile([128, NT, 32], f32, tag="otile", name="otile")
        obufs = [apl.tile([128, 8, 128], f32, tag=f"obuf{i}", name=f"obuf{i}")
                 for i in range(2)]
        # zero only the padding borders (never rewritten), on gpsimd
        for c1o_i in c1os:
            nc.gpsimd.memset(mk(c1o_i, 0, [(100, NT), (90, 2), (1, 10)]), 0.0)
            nc.gpsimd.memset(mk(c1o_i, 10, [(100, NT), (10, 8), (9, 2)]), 0.0)
        nc.gpsimd.memset(mk(c2o, 0, [(180, NT), (170, 2), (1, 10)]), 0.0)
        # odd-pair half-pads: slot 0 (dj=1 half) and slot 8 (dj=0 half) stay 0
        nc.gpsimd.memset(mk(c2o, 10, [(180, NT), (10, 16), (8, 2)]), 0.0)
        nc.gpsimd.memset(mk(c3o, 0, [(272, NT), (34, 8), (33, 2)]), 0.0)

        def conv1_tile(t):
            # conv1: psum [c128, (n32,4,4)] -> c1os[t % 2]
            c1o = c1os[t % 2]
            ns = t * NT
            for pi4, (di, dj) in enumerate([(0, 0), (0, 1), (1, 0), (1, 1)]):
                ps = psum()
                tn = 0
                for si in _sis(di):
                    for sj in _sis(dj):
                        for h in range(2):
                            rhs = mk(c1in[h], ns * 36 + (1 + si) * 6 + (1 + sj),
                                     [(36, NT), (6, 4), (1, 4)])
                            nc.tensor.matmul(ps[:, 0:512], k1s[:, pi4, tn, h, :], rhs,
                                             start=(tn == 0 and h == 0),
                                             stop=(tn == 3 and h == 1))
                        tn += 1
                off1 = (1 + di) * 10 + (1 + dj)
                nc.scalar.activation(mk(c1o, off1, [(100, 16), (20, 4), (2, 4)]),
                                     ps[:, 0:256], AF.Relu, bias=b1[:])
                nc.vector.tensor_scalar(
                    mk(c1o, off1 + 16 * 100, [(100, 16), (20, 4), (2, 4)]),
                    ps[:, 256:512], b1[:], 0.0, op0=ALU.add, op1=ALU.max)

        def _epilogue_half(t, g):
            obuf = obufs[t % 2]
            pst = psum()
            for cc in range(4):
                cch = g * 4 + cc
                nc.tensor.transpose(pst[0:128, cc * 128:(cc + 1) * 128],
                                    otile[:, cch * 4:cch * 4 + 4, :].bitcast(f32),
                                    ident[:])
            nc.vector.tensor_copy(obuf[:, g * 4:(g + 1) * 4, :], pst[:, 0:512])
            dst = bass.AP(tensor=out_d.tensor,
                          offset=out_d.offset + t * NT * 4096 + g * 4 * 16384,
                          ap=[[4096, 4], [128, 32], [16384, 4], [1, 128]])
            nc.sync.dma_start(out=dst, in_=obuf[:, g * 4:(g + 1) * 4, :])

        conv1_tile(0)
        for t in range(NPC // NT):
            ns = t * NT
            c1o = c1os[t % 2]
            # conv2: psum cols (n8, i8, q'7) compute odd output pairs with
            # zero-waste 4-accum chains; edge cols 448:512 produce j=0/j=15
            for di in range(2):
                for c4 in range(4):
                    ps2 = psum()
                    mm = 0
                    for ii, si in enumerate(_sis(di)):
                        for p in range(2):
                            rhs = mk(c1o, (c4 * 8) * 100 + (1 + si) * 10 + 1 + p,
                                     [(100, 8), (10, 8), (1, 7)])
                            nc.tensor.matmul(ps2[:, 0:448],
                                             k2s[:, di, ii, p, :], rhs,
                                             start=(mm == 0), stop=(mm == 3))
                            mm += 1
                    for e in range(2):
                        for me, (ii, si) in enumerate(enumerate(_sis(di))):
                            rhs = mk(c1o,
                                     (c4 * 8) * 100 + (1 + si) * 10 + (1, 8)[e],
                                     [(100, 8), (10, 8)])
                            nc.tensor.matmul(
                                ps2[e * 64:(e + 1) * 64, 448:512],
                                k2e[:, di, ii, e, :], rhs,
                                start=(me == 0), stop=(me == 1))
                    off2 = (c4 * 8) * 180 + (1 + di) * 10
                    # main: sigma 1..7, both partition halves, one act
                    dstm = mk(c2o, off2 + 1, [(180, 8), (20, 8), (1, 7)])
                    if c4 % 2 == 0:
                        nc.scalar.activation(dstm, ps2[:, 0:448], AF.Relu,
                                             bias=b2[:])
                    else:
                        nc.vector.tensor_scalar(dstm, ps2[:, 0:448], b2[:], 0.0,
                                                op0=ALU.add, op1=ALU.max)
                    # edges: j0 -> sigma0 dj0-half, j15 -> sigma8 dj1-half
                    nc.vector.tensor_scalar(
                        mkh(c2o, 0, 64, off2, [(180, 8), (20, 8)]),
                        ps2[0:64, 448:512], b2[0:64, 0:1], 0.0,
                        op0=ALU.add, op1=ALU.max)
                    nc.scalar.activation(
                        mkh(c2o, 64, 64, off2 + 8, [(180, 8), (20, 8)]),
                        ps2[64:128, 448:512], AF.Relu, bias=b2[64:128, 0:1])
            # conv3: psum [(jm4,c32), (n4,16,qp8)]; fill each bank
            # consecutively, drain it while the next bank fills. grp outer so
            # the first conv4 bank's act dependencies retire first. On the
            # last tile run grp1/bank-B first: each bank's deps then drain
            # under the other group's fills (no conv1(t+1) cover exists).
            last = (t == NPC // NT - 1)
            grp_order = (1, 0) if last else (0, 1)
            for grp in grp_order:
                for di in range(2):
                    for cg in range(4):
                        c8 = grp * 4 + cg
                        ps3 = psum()
                        mm = 0
                        for ii, si in enumerate(_sis(di)):
                            for delta in range(2):
                                rhs = mk(c2o, (c8 * 4) * 180 + (1 + si) * 10 + delta,
                                         [(180, 4), (10, 16), (1, 8)])
                                nc.tensor.matmul(ps3[:, 0:512],
                                                 k3s[:, di, ii, delta, :], rhs,
                                                 start=(mm == 0), stop=(mm == 3))
                                mm += 1
                        off3 = (c8 * 4) * 272 + (1 + di)
                        if last and grp == grp_order[1]:
                            # final group of the run: 4-way split halves the
                            # queue tail that gates the last conv4 bank
                            for pi3, i0 in enumerate((0, 4, 8, 12)):
                                dst3 = mk(c3o, off3 + 2 * i0,
                                          [(272, 4), (2, 4), (34, 8)])
                                src3 = mk(ps3, 8 * i0, [(128, 4), (1, 32)])
                                if pi3 % 2 == 0:
                                    nc.scalar.activation(dst3, src3, AF.Relu,
                                                         bias=b3[:])
                                else:
                                    nc.vector.tensor_scalar(
                                        dst3, src3, b3[:], 0.0,
                                        op0=ALU.add, op1=ALU.max)
                        else:
                            # split cols i (scalar/vector) per n; last tile
                            # balances 8/8 so neither queue tail gates conv4
                            ns3 = 8 if last else 7
                            nc.scalar.activation(
                                mk(c3o, off3, [(272, 4), (2, ns3), (34, 8)]),
                                mk(ps3, 0, [(128, 4), (1, 8 * ns3)]), AF.Relu,
                                bias=b3[:])
                            nc.vector.tensor_scalar(
                                mk(c3o, off3 + 2 * ns3,
                                   [(272, 4), (2, 16 - ns3), (34, 8)]),
                                mk(ps3, 8 * ns3,
                                   [(128, 4), (1, 8 * (16 - ns3))]),
                                b3[:], 0.0, op0=ALU.add, op1=ALU.max)
            # next tile's conv1 here: its fills cover the conv3 act drain and
            # its acts retire under conv4/transposes below
            if t + 1 < NPC // NT:
                conv1_tile(t + 1)
            # conv4: psum [(di,dj,jout), (n16,32)]; first bank's 24 accums
            # overlap the second conv3 group's act drain
            c2c_order = (1, 0) if last else (0, 1)
            for ci, c2c in enumerate(c2c_order):
                ps4 = psum()
                mm = 0
                for bblk in range(8):
                    for ii in range(3):
                        si = ii - 1
                        rhs = mk(c3o, (c2c * 16) * 272 + bblk * 34 + (1 + si),
                                 [(272, 16), (1, 32)])
                        nc.tensor.matmul(ps4[:, 0:512], k4s[:, bblk, ii, :], rhs,
                                         start=(mm == 0), stop=(mm == 23))
                        mm += 1
                nc.scalar.activation(mk(otile, (c2c * 16) * 32, [(32, 10), (1, 32)]),
                                     ps4[:, 0:320], AF.Identity, bias=b4[:])
                nc.vector.tensor_scalar_add(
                    mk(otile, (c2c * 16 + 10) * 32, [(32, 6), (1, 32)]),
                    ps4[:, 320:512], b4[:])
                if last and ci == 0:
                    # drain the first-processed half under the other bank's
                    # accumulation
                    _epilogue_half(t, c2c)
            # transpose [m=(di,x), (n,io)] -> [(n4,io32), 128 contiguous pixels]
            if last:
                _epilogue_half(t, c2c_order[1])
            else:
                obuf = obufs[t % 2]
                for g in range(2):
                    pst = psum()
                    for cc in range(4):
                        cch = g * 4 + cc
                        nc.tensor.transpose(pst[0:128, cc * 128:(cc + 1) * 128],
                                            otile[:, cch * 4:cch * 4 + 4, :].bitcast(f32),
                                            ident[:])
                    nc.vector.tensor_copy(obuf[:, g * 4:(g + 1) * 4, :], pst[:, 0:512])
                dst = bass.AP(tensor=out_d.tensor, offset=out_d.offset + ns * 4096,
                              ap=[[4096, 4], [128, 32], [16384, 8], [1, 128]])
                nc.sync.dma_start(out=dst, in_=obuf[:])
        ctx.close()
    nc.compile()
    return nc


_NC_CACHE = {}


def kernel(angles, item_rep, W, b, k1, b1, k2, b2, k3, b3, k4, b4):
    _install_axon_shim()
    from concourse.bass_utils import run_bass_kernel_spmd
    if 'nc' not in _NC_CACHE:
        _NC_CACHE['nc'] = _build()
    nc = _NC_CACHE['nc']
    wts = _prep_weights(np.asarray(item_rep, np.float32), np.asarray(W, np.float32),
                        np.asarray(b, np.float32), np.asarray(k1, np.float32),
                        np.asarray(b1, np.float32), np.asarray(k2, np.float32),
                        np.asarray(b2, np.float32), np.asarray(k3, np.float32),
                        np.asarray(b3, np.float32), np.asarray(k4, np.float32),
                        np.asarray(b4, np.float32))
    angles = np.asarray(angles, np.float32)
    in_maps = []
    for c in range(NCORES):
        m = dict(wts)
        m['angles'] = np.ascontiguousarray(angles[c * NPC:(c + 1) * NPC])
        in_maps.append(m)
    res = run_bass_kernel_spmd(nc, in_maps, core_ids=list(range(NCORES)))
    return np.concatenate([r['out'][:, None, :, :] for r in res.results], axis=0)

